# revision 10
# baseline (speedup 1.0000x reference)
"""LocallyConnected1d Trainium2 kernel (8 NeuronCores, SPMD).

Problem (hardcoded): x [128, 64, 1028] f32, weight [1, 64, 64, 256, 8] f32,
out[b, c, o] = sum_{ci,k} x[b, ci, 4*o + k] * w[c, ci, o, k] / sqrt(64),
out shape [128, 64, 256] f32.  O=256, K=8, S=4.

Strategy:
  - 2D sharding over 8 cores: B split 2 x Co split 4 -> per-core shard
    B_LOC=64, C_LOC=16.  Minimizes per-core HBM bytes (x/2 + w/4).
  - bf16 operands, fp32 PSUM accumulation.  /sqrt(64) folded into weights.
  - Per output position o: 4 accumulating matmuls of [128, C_LOC]
    (stationary W) x [128, B_LOC] (moving X).  Contraction rows ordered
    (k_hi, ci_half, ci_in, k_lo) so the moving operand is a pure
    *reshape* of x (no unfold duplication): with k = 4*k_hi + k_lo,
    x[b, ci, 4*o+k] = G[(ci, k_lo), o + k_hi, b] where
    G[(ci, k_lo), t, b] = x[b, ci, 4*t + k_lo].
  - Fine-grained pipeline: G in 16 t-chunks (17 cols, 1 overlap) on the
    SP HWDGE queue; W in 8 o-chunks on the ACT HWDGE queue; PE compute
    trails the DMA stream chunk-by-chunk so the PE never starves long
    (HAM stays warm); outputs trickle out on the gpsimd SWDGE queue.
"""

import sys

for _p in ("/opt/trn_rl_repo",):
    if _p not in sys.path:
        sys.path.insert(0, _p)

import numpy as np
import ml_dtypes

B, CI, CO, O, K, S = 128, 64, 64, 256, 8, 4
L = 1028
P_B, Q_C = 2, 4                      # B-split x Co-split = 8 cores
B_LOC, C_LOC = B // P_B, CO // Q_C   # 64, 16
NCH_G = 16                           # G t-chunks
OCH_G = O // NCH_G                   # 16 positions per G chunk
TCH = OCH_G + 1                      # 17 t-cols per chunk (overlap 1)
NCH_W = 8                            # W o-chunks
OCH_W = O // NCH_W                   # 32 positions per W chunk
GRP = 16                             # o-positions per PSUM bank group
N_CORES = 8

_prog_cache = {}


def _build_program():
    if "nc" in _prog_cache:
        return _prog_cache["nc"]
    import concourse.tile as tile
    from concourse import bacc, mybir

    bf16 = mybir.dt.bfloat16
    f32 = mybir.dt.float32

    nc = bacc.Bacc("TRN2", target_bir_lowering=False, debug=False,
                   num_devices=N_CORES)
    xg = nc.dram_tensor("xg", [NCH_G, 128, 2 * TCH * B_LOC], bf16,
                        kind="ExternalInput").ap()
    wt = nc.dram_tensor("wt", [NCH_W, 128, OCH_W * 4 * C_LOC], bf16,
                        kind="ExternalInput").ap()
    out = nc.dram_tensor("out", [C_LOC, O * B_LOC], f32,
                         kind="ExternalOutput").ap()

    with tile.TileContext(nc) as tc:
        with (
            tc.tile_pool(name="g", bufs=1) as gpool,
            tc.tile_pool(name="w", bufs=1) as wpool,
            tc.tile_pool(name="ps", bufs=3, space="PSUM") as pspool,
            tc.tile_pool(name="ob", bufs=6) as opool,
        ):
            gts, wts = {}, {}

            # PE warm-up: ~2.5us of dummy matmuls while the first G/W
            # chunks are still in flight, so HAM un-throttles the PE
            # clock (1.2 -> 2.4 GHz) before the real stream starts.
            wu = gpool.tile([128, 256], bf16, tag="warm")
            nc.vector.memset(wu[:], 0.0)
            with tc.tile_pool(name="wps", bufs=1, space="PSUM") as wpspool:
                wps = wpspool.tile([C_LOC, 256], f32, tag="warmps")
                for _ in range(12):
                    nc.tensor.matmul(wps[:, :256], wu[:, :C_LOC],
                                     wu[:, :256], start=True, stop=True)

            qrr = [0]

            def _q():
                qrr[0] ^= 1
                return nc.sync if qrr[0] else nc.scalar

            def load_g(c):
                g = gpool.tile([128, 2 * TCH * B_LOC], bf16, tag=f"g{c}")
                _q().dma_start(g[:], xg[c])
                gts[c] = g

            def load_w(c):
                w = wpool.tile([128, OCH_W * 4 * C_LOC], bf16, tag=f"w{c}")
                _q().dma_start(w[:], wt[c])
                wts[c] = w

            def compute_chunk(cg):
                """One G-chunk = 16 o's = 2 PSUM groups of 8."""
                cw = cg * OCH_G // OCH_W        # map G chunk -> W chunk
                for grp in range(OCH_G // GRP):
                    ps = pspool.tile([C_LOC, GRP * B_LOC], f32)
                    for oo in range(GRP):
                        o = cg * OCH_G + grp * GRP + oo
                        og = o - cg * OCH_G          # within G chunk
                        ow = o - cw * OCH_W          # within W chunk
                        for j in range(4):
                            khi, h = j // 2, j % 2
                            wcol = (ow * 4 + khi * 2 + h) * C_LOC
                            gcol = (h * TCH + og + khi) * B_LOC
                            nc.tensor.matmul(
                                ps[:, oo * B_LOC:(oo + 1) * B_LOC],
                                wts[cw][:, wcol:wcol + C_LOC],
                                gts[cg][:, gcol:gcol + B_LOC],
                                start=(j == 0), stop=(j == 3),
                            )
                    ob = opool.tile([C_LOC, GRP * B_LOC], f32)
                    nc.vector.tensor_copy(ob[:], ps[:])
                    o0 = cg * OCH_G + grp * GRP
                    nc.gpsimd.dma_start(
                        out[:, o0 * B_LOC:(o0 + GRP) * B_LOC], ob[:])

            # Prefetch: 2 G chunks + 1 W chunk ahead of compute.
            load_w(0)
            load_g(0)
            load_g(1)
            load_w(1)
            for cg in range(NCH_G):
                # prefetch next G chunk and (when crossing) next W chunk
                ng = cg + 2
                if ng < NCH_G:
                    load_g(ng)
                    nw = (ng * OCH_G) // OCH_W + 1
                    if nw < NCH_W and nw not in wts and \
                            (ng * OCH_G) % OCH_W == 0:
                        load_w(nw)
                compute_chunk(cg)

    nc.compile()
    _prog_cache["nc"] = nc
    return nc


def _shard_inputs(x, weight):
    """Host-side shard + relayout.  Returns in_maps for the 8 cores."""
    bf16 = ml_dtypes.bfloat16
    w0 = (np.asarray(weight, np.float32)[0] / 8.0)     # [Co, Ci, O, K]
    x = np.asarray(x, np.float32)
    in_maps = []
    for r in range(N_CORES):
        b0 = (r // Q_C) * B_LOC
        c0 = (r % Q_C) * C_LOC
        # G: [ci, klo, t, b] rows=(ci_in*4+klo), h=ci//32
        arr = x[b0:b0 + B_LOC].reshape(B_LOC, CI, L // 4, 4)
        arr = arr.transpose(1, 3, 2, 0).reshape(2, 128, L // 4, B_LOC)
        arr = arr.astype(bf16)
        g_chunks = np.empty((NCH_G, 128, 2 * TCH * B_LOC), bf16)
        for c in range(NCH_G):
            gc = arr[:, :, OCH_G * c:OCH_G * c + TCH, :].reshape(
                2, 128, TCH * B_LOC)
            g_chunks[c] = np.concatenate([gc[0], gc[1]], axis=1)
        # W: rows=(ci_in*4+klo); free = o_loc*64 + khi*32 + h*16 + c
        wv = w0[c0:c0 + C_LOC].reshape(C_LOC, 2, 32, O, 2, 4)
        wv = wv.transpose(2, 5, 3, 4, 1, 0).reshape(128, O * 4 * C_LOC)
        wv = np.ascontiguousarray(wv).astype(bf16)
        w_chunks = wv.reshape(128, NCH_W, OCH_W * 4 * C_LOC).transpose(1, 0, 2)
        w_chunks = np.ascontiguousarray(w_chunks)
        in_maps.append({"xg": g_chunks, "wt": w_chunks})
    return in_maps


def _gather(results):
    out_full = np.empty((B, CO, O), np.float32)
    for r in range(N_CORES):
        b0 = (r // Q_C) * B_LOC
        c0 = (r % Q_C) * C_LOC
        sh = results[r]["out"].reshape(C_LOC, O, B_LOC)
        out_full[b0:b0 + B_LOC, c0:c0 + C_LOC, :] = sh.transpose(2, 0, 1)
    return out_full


def kernel(x, weight):
    from concourse.bass_utils import run_bass_kernel_spmd
    nc = _build_program()
    in_maps = _shard_inputs(x, weight)
    res = run_bass_kernel_spmd(nc, in_maps, list(range(N_CORES)))
    return _gather(res.results)
